# revision 1
# baseline (speedup 1.0000x reference)
"""Trainium2 Bass kernel for nn_GATSampling (2-layer bipartite GAT, 8 NeuronCores).

Strategy (SPMD over 8 cores, host re-shards between launches):
  Launch T: distributed feature transform. Each core transforms 1/8 of feat0
            rows: fs0 = feat0 @ [W0 | W0@al0_mask]  -> [rows, 132]
            and 1/8 of feat1 rows: er0 = feat1 @ (W0@ar0_mask) -> [rows, 4].
  Host:     destination nodes of layer 0 are dealt (degree-balanced) into
            392 blocks of <=128 dst slots (49 blocks/core). Edges are sorted
            by (block, slot) and padded to K0 chunks of 128 edges per block.
            A per-edge stream row [fs0[src] | el0[src] | er0[dst]] (136 f32)
            is assembled per core.
  Launch A: layer-0 edge phase per block: s = exp(clamp(leakyrelu(el+er))),
            m = fs * s, segment-sum via one-hot matmul (S^T @ [m|s]) into
            PSUM, normalization by 1/outsum folded out of the edge loop,
            ELU, then h1ext = elu(h1) @ [W1 | W1@al1_mask | W1@ar1_mask].
  Host:     scatter h1ext rows back to global [50000, 136]; build layer-2
            stream rows [fs1[src] | el1[src] | er1g[map12[dst]]].
  Launch B: layer-2 edge phase (same structure, no ELU), mean over heads
            folded into the 1/outsum scaling; output logits.
  Host:     scatter logits to [12500, 32].

Everything numeric except index bookkeeping and the tiny (128x132) weight
products runs on the NeuronCores in f32.
"""
import sys

sys.path.insert(0, "/opt/trn_rl_repo")

import numpy as np

from concourse import bass, mybir, tile, bacc, bass_utils

F32 = mybir.dt.float32
P = 128
NCORES = 8
NEG_SLOPE = 0.2
H, D = 4, 32
HD = H * D  # 128

# problem sizes (hardcoded per spec)
N0, N1, N2 = 200000, 50000, 12500
E0, E1 = 800000, 200000
F_IN = 128

# layer-0 transform tiling
T0_CHUNKS = -(-N0 // (NCORES * P))        # 196 chunks of feat0 per core
T0_ROWS = T0_CHUNKS * P                   # 25088
T1_CHUNKS = -(-N1 // (NCORES * P))        # 49 chunks of feat1 per core
T1_ROWS = T1_CHUNKS * P                   # 6272

# edge-phase blocks
NBLK0 = 49                                # layer-0 dst blocks per core
NBLK1 = 13                                # layer-2 dst blocks per core

_IOTA = np.broadcast_to(np.arange(P, dtype=np.float32), (P, P)).copy()
_IDENT = np.eye(P, dtype=np.float32)

_cache = {}


# --------------------------------------------------------------------------
# host-side graph preprocessing
# --------------------------------------------------------------------------
def _deal_blocks(dst, n_dst, nblocks):
    """Deal destination nodes into `nblocks` blocks of <=128 slots,
    balancing edge counts. Returns (slot_of_dst [n_dst] -> global slot id,
    per-block edge lists as a permutation of edges sorted by (block, slot),
    K = max chunks per block)."""
    deg = np.bincount(dst, minlength=n_dst)
    order = np.argsort(-deg, kind="stable")
    # round-robin deal: dst order[i] -> block i % nblocks, slot i // nblocks
    blk = np.empty(n_dst, np.int64)
    slot_in_blk = np.empty(n_dst, np.int64)
    blk[order] = np.arange(n_dst) % nblocks
    slot_in_blk[order] = np.arange(n_dst) // nblocks
    assert slot_in_blk.max() < P, "block slot overflow"
    slot_of_dst = blk * P + slot_in_blk
    # sort edges by (block, slot)
    eslot = slot_of_dst[dst]
    eorder = np.argsort(eslot, kind="stable")
    blk_edge_counts = np.bincount(blk[dst], minlength=nblocks)
    K = int(-(-blk_edge_counts.max() // P))
    return slot_of_dst, eorder, blk_edge_counts, K


def _build_stream(rows_src, er_rows, dst_slots, eorder, blk_counts, nblocks, K):
    """Assemble the padded per-edge stream and dstr for ALL cores at once.

    rows_src: [E, 132] f32 rows ([fs|el]) already gathered per edge (sorted order
              applied by caller through eorder indexing here).
    er_rows:  [E, 4] per-edge er values (same edge order as rows_src).
    dst_slots: [E] global slot id per edge (unsorted).
    Returns stream [NCORES, 128, C, 136], dstr [NCORES, 128, C] with
    C = (nblocks // NCORES) * K.
    """
    nblk_core = nblocks // NCORES
    C = nblk_core * K
    Epad = nblocks * K * P
    stream_flat = np.zeros((Epad, 136), np.float32)
    stream_flat[:, 132:136] = -1e30
    dstr_flat = np.zeros(Epad, np.float32)

    # position of each sorted edge within the padded layout
    starts = np.zeros(nblocks + 1, np.int64)
    np.cumsum(blk_counts, out=starts[1:])
    sorted_slots = dst_slots[eorder]
    sorted_blk = sorted_slots // P
    # offset of edge within its block
    within = np.arange(len(eorder)) - starts[sorted_blk]
    pos = sorted_blk * (K * P) + within
    stream_flat[pos, 0:132] = rows_src[eorder]
    stream_flat[pos, 132:136] = er_rows[eorder]
    dstr_flat[pos] = (sorted_slots % P).astype(np.float32)

    # reshape to [NCORES, 128, C, 136]: block b (global) -> core b // nblk_core
    # edge within block: chunk k = within // P, lane p = within % P
    s = stream_flat.reshape(NCORES, nblk_core, K, P, 136)
    stream = np.ascontiguousarray(s.transpose(0, 3, 1, 2, 4)).reshape(
        NCORES, P, C, 136)
    d = dstr_flat.reshape(NCORES, nblk_core, K, P)
    dstr = np.ascontiguousarray(d.transpose(0, 3, 1, 2)).reshape(NCORES, P, C)
    return stream, dstr


# --------------------------------------------------------------------------
# bass programs
# --------------------------------------------------------------------------
def _build_T(repeat=1):
    nc = bacc.Bacc("TRN2", target_bir_lowering=False, debug=False)
    f0 = nc.dram_tensor("f0", [T0_ROWS, F_IN], F32, kind="ExternalInput").ap()
    f1 = nc.dram_tensor("f1", [T1_ROWS, F_IN], F32, kind="ExternalInput").ap()
    w0full = nc.dram_tensor("w0full", [F_IN, 132], F32, kind="ExternalInput").ap()
    w0ar = nc.dram_tensor("w0ar", [F_IN, 4], F32, kind="ExternalInput").ap()
    ident = nc.dram_tensor("ident", [P, P], F32, kind="ExternalInput").ap()
    fs0 = nc.dram_tensor("fs0", [T0_ROWS, 132], F32, kind="ExternalOutput").ap()
    er0 = nc.dram_tensor("er0", [T1_ROWS, 4], F32, kind="ExternalOutput").ap()

    with tile.TileContext(nc) as tc:
        with (
            tc.tile_pool(name="const", bufs=1) as cpool,
            tc.tile_pool(name="load", bufs=3) as lpool,
            tc.tile_pool(name="work", bufs=3) as wpool,
            tc.tile_pool(name="ps", bufs=4, space="PSUM") as ppool,
        ):
            ident_sb = cpool.tile([P, P], F32)
            nc.sync.dma_start(ident_sb[:], ident)
            w0full_sb = cpool.tile([F_IN, 132], F32)
            nc.sync.dma_start(w0full_sb[:], w0full)
            w0ar_sb = cpool.tile([F_IN, 4], F32)
            nc.sync.dma_start(w0ar_sb[:], w0ar)

            def transform(src_d, dst_d, nchunks, w_sb, ncols):
                for i in range(nchunks):
                    ch = lpool.tile([P, F_IN], F32, tag="ch")
                    nc.sync.dma_start(ch[:], src_d[i * P:(i + 1) * P, :])
                    pst = ppool.tile([P, P], F32, space="PSUM", tag="pst")
                    nc.tensor.transpose(out=pst[:], in_=ch[:],
                                        identity=ident_sb[:])
                    chT = wpool.tile([P, P], F32, tag="chT")
                    nc.scalar.copy(chT[:], pst[:])
                    ps2 = ppool.tile([P, ncols], F32, space="PSUM", tag="ps2")
                    nc.tensor.matmul(ps2[:], lhsT=chT[:], rhs=w_sb[:],
                                     start=True, stop=True)
                    osb = wpool.tile([P, ncols], F32, tag="osb")
                    nc.scalar.copy(osb[:], ps2[:])
                    nc.sync.dma_start(dst_d[i * P:(i + 1) * P, :], osb[:])

            for _rep in range(repeat):
                transform(f0, fs0, T0_CHUNKS, w0full_sb, 132)
                transform(f1, er0, T1_CHUNKS, w0ar_sb, 4)

    nc.compile()
    return nc


def _build_edge_phase(K, nblk, out_transform, repeat=1):
    """Edge-phase program. If out_transform, apply ELU + @W1full (layer 0,
    outputs [nblk*128, 136]); else mean-over-heads logits ([nblk*128, 32])."""
    C = nblk * K
    nc = bacc.Bacc("TRN2", target_bir_lowering=False, debug=False)
    stream_d = nc.dram_tensor("stream", [P, C, 136], F32, kind="ExternalInput").ap()
    dstr_d = nc.dram_tensor("dstr", [P, C], F32, kind="ExternalInput").ap()
    iota_d = nc.dram_tensor("iota", [P, P], F32, kind="ExternalInput").ap()
    if out_transform:
        w1_d = nc.dram_tensor("w1full", [128, 136], F32, kind="ExternalInput").ap()
        ident_d = nc.dram_tensor("ident", [P, P], F32, kind="ExternalInput").ap()
        out_d = nc.dram_tensor("out", [nblk * P, 136], F32, kind="ExternalOutput").ap()
    else:
        out_d = nc.dram_tensor("out", [nblk * P, 32], F32, kind="ExternalOutput").ap()

    with tile.TileContext(nc) as tc:
        with (
            tc.tile_pool(name="const", bufs=1) as cpool,
            tc.tile_pool(name="gload", bufs=3) as gpool,
            tc.tile_pool(name="sgen", bufs=4) as spool,
            tc.tile_pool(name="work", bufs=3) as wpool,
            tc.tile_pool(name="ps", bufs=2, space="PSUM") as ppool,
            tc.tile_pool(name="ps2", bufs=2, space="PSUM") as ppool2,
        ):
            iota_sb = cpool.tile([P, P], F32)
            nc.sync.dma_start(iota_sb[:], iota_d)
            dstr_sb = cpool.tile([P, C], F32)
            nc.sync.dma_start(dstr_sb[:], dstr_d)
            if out_transform:
                ident_sb = cpool.tile([P, P], F32)
                nc.sync.dma_start(ident_sb[:], ident_d)
                w1_sb = cpool.tile([128, 136], F32)
                nc.sync.dma_start(w1_sb[:], w1_d)

            for _rep, b in ((r, bb) for r in range(repeat)
                            for bb in range(nblk)):
                G = gpool.tile([P, K, 136], F32, tag="G")
                nc.sync.dma_start(G[:], stream_d[:, b * K:(b + 1) * K, :])

                # s = exp(max(leaky(el + er), -80)) -> into el slot
                et = wpool.tile([P, K, 4], F32, tag="et")
                nc.vector.tensor_tensor(out=et[:], in0=G[:, :, 128:132],
                                        in1=G[:, :, 132:136],
                                        op=mybir.AluOpType.add)
                lk = wpool.tile([P, K, 4], F32, tag="lk")
                nc.vector.tensor_scalar(out=lk[:], in0=et[:], scalar1=NEG_SLOPE,
                                        scalar2=None, op0=mybir.AluOpType.mult)
                nc.vector.tensor_tensor(out=et[:], in0=et[:], in1=lk[:],
                                        op=mybir.AluOpType.max)
                nc.vector.tensor_scalar(out=et[:], in0=et[:], scalar1=-80.0,
                                        scalar2=None, op0=mybir.AluOpType.max)
                nc.scalar.activation(out=G[:, :, 128:132], in_=et[:],
                                     func=mybir.ActivationFunctionType.Exp)

                # m = fs * s for the whole block in one broadcast multiply
                fs_blk = G[:, :, 0:128].rearrange("p k (h d) -> p k h d", h=H)
                s_blk = G[:, :, 128:132].unsqueeze(3).to_broadcast([P, K, H, D])
                nc.vector.tensor_tensor(out=fs_blk, in0=fs_blk, in1=s_blk,
                                        op=mybir.AluOpType.mult)

                psum = ppool.tile([P, 132], F32, space="PSUM", tag="ps")
                for k in range(K):
                    c = b * K + k
                    S = spool.tile([P, P], F32, tag="S")
                    eng = nc.gpsimd if k % 3 == 0 else nc.vector
                    eng.tensor_scalar(
                        out=S[:], in0=iota_sb[:], scalar1=dstr_sb[:, c:c + 1],
                        scalar2=None, op0=mybir.AluOpType.is_equal)
                    nc.tensor.matmul(psum[:], lhsT=S[:], rhs=G[:, k, 0:132],
                                     start=(k == 0), stop=(k == K - 1))

                rec = wpool.tile([P, 4], F32, tag="rec")
                nc.vector.tensor_scalar(out=rec[:], in0=psum[:, 128:132],
                                        scalar1=1e-30, scalar2=None,
                                        op0=mybir.AluOpType.add)
                nc.vector.reciprocal(rec[:], rec[:])

                if out_transform:
                    rst = wpool.tile([P, 128], F32, tag="rst")
                    for h in range(H):
                        nc.vector.tensor_scalar(
                            out=rst[:, h * D:(h + 1) * D],
                            in0=psum[:, h * D:(h + 1) * D],
                            scalar1=rec[:, h:h + 1], scalar2=None,
                            op0=mybir.AluOpType.mult)
                    # elu = exp(min(x,0)) - 1 + max(x,0)
                    mn = wpool.tile([P, 128], F32, tag="mn")
                    nc.gpsimd.tensor_scalar(out=mn[:], in0=rst[:], scalar1=0.0,
                                            scalar2=None, op0=mybir.AluOpType.min)
                    ex = wpool.tile([P, 128], F32, tag="ex")
                    nc.scalar.activation(out=ex[:], in_=mn[:],
                                         func=mybir.ActivationFunctionType.Exp)
                    mx = wpool.tile([P, 128], F32, tag="mx")
                    nc.gpsimd.tensor_scalar(out=mx[:], in0=rst[:], scalar1=0.0,
                                            scalar2=None, op0=mybir.AluOpType.max)
                    elu = wpool.tile([P, 128], F32, tag="elu")
                    nc.gpsimd.tensor_tensor(out=elu[:], in0=ex[:], in1=mx[:],
                                            op=mybir.AluOpType.add)
                    nc.gpsimd.tensor_scalar(out=elu[:], in0=elu[:], scalar1=1.0,
                                            scalar2=None,
                                            op0=mybir.AluOpType.subtract)
                    pst = ppool2.tile([P, P], F32, space="PSUM", tag="pst")
                    nc.tensor.transpose(out=pst[:], in_=elu[:],
                                        identity=ident_sb[:])
                    eluT = wpool.tile([P, P], F32, tag="eluT")
                    nc.scalar.copy(eluT[:], pst[:])
                    ps2 = ppool2.tile([P, 136], F32, space="PSUM", tag="ps2")
                    nc.tensor.matmul(ps2[:], lhsT=eluT[:], rhs=w1_sb[:],
                                     start=True, stop=True)
                    osb = wpool.tile([P, 136], F32, tag="osb")
                    nc.scalar.copy(osb[:], ps2[:])
                    nc.sync.dma_start(out_d[b * P:(b + 1) * P, :], osb[:])
                else:
                    # logits = 0.25 * sum_h msum[:, h] * rec[:, h]
                    rec2 = wpool.tile([P, 4], F32, tag="rec2")
                    nc.vector.tensor_scalar(out=rec2[:], in0=rec[:],
                                            scalar1=0.25, scalar2=None,
                                            op0=mybir.AluOpType.mult)
                    acc = wpool.tile([P, 32], F32, tag="acc")
                    tmp = wpool.tile([P, 32], F32, tag="tmp")
                    nc.vector.tensor_scalar(
                        out=acc[:], in0=psum[:, 0:D],
                        scalar1=rec2[:, 0:1], scalar2=None,
                        op0=mybir.AluOpType.mult)
                    for h in range(1, H):
                        nc.vector.tensor_scalar(
                            out=tmp[:], in0=psum[:, h * D:(h + 1) * D],
                            scalar1=rec2[:, h:h + 1], scalar2=None,
                            op0=mybir.AluOpType.mult)
                        nc.vector.tensor_tensor(out=acc[:], in0=acc[:],
                                                in1=tmp[:],
                                                op=mybir.AluOpType.add)
                    nc.sync.dma_start(out_d[b * P:(b + 1) * P, :], acc[:])

    nc.compile()
    return nc


def _get_programs(K0, K1):
    key = (K0, K1)
    if key not in _cache:
        _cache[key] = (
            _build_T(),
            _build_edge_phase(K0, NBLK0, True),
            _build_edge_phase(K1, NBLK1, False),
        )
    return _cache[key]


def _run(nc, in_maps, trace=False):
    return bass_utils.run_bass_kernel_spmd(
        nc, in_maps, list(range(NCORES)), trace=trace)


# --------------------------------------------------------------------------
# main entry
# --------------------------------------------------------------------------
def kernel(feat0, feat1, src0, dst0, src1, dst1, map12,
           W0, al0, ar0, W1, al1, ar1, _collect_times=None, _trace=False):
    feat0 = np.asarray(feat0)
    feat1 = np.asarray(feat1)
    src0 = np.asarray(src0).astype(np.int64)
    dst0 = np.asarray(dst0).astype(np.int64)
    src1 = np.asarray(src1).astype(np.int64)
    dst1 = np.asarray(dst1).astype(np.int64)
    map12 = np.asarray(map12).astype(np.int64)
    W0 = np.asarray(W0); al0 = np.asarray(al0); ar0 = np.asarray(ar0)
    W1 = np.asarray(W1); al1 = np.asarray(al1); ar1 = np.asarray(ar1)

    # tiny weight products (host)
    al0m = np.zeros((HD, H), np.float32)
    ar0m = np.zeros((HD, H), np.float32)
    al1m = np.zeros((HD, H), np.float32)
    ar1m = np.zeros((HD, H), np.float32)
    for h in range(H):
        al0m[h * D:(h + 1) * D, h] = al0[h]
        ar0m[h * D:(h + 1) * D, h] = ar0[h]
        al1m[h * D:(h + 1) * D, h] = al1[h]
        ar1m[h * D:(h + 1) * D, h] = ar1[h]
    W0full = np.concatenate([W0, W0 @ al0m], axis=1).astype(np.float32)
    W0ar = (W0 @ ar0m).astype(np.float32)
    W1full = np.concatenate([W1, W1 @ al1m, W1 @ ar1m], axis=1).astype(np.float32)

    # graph partitioning (host, index-only)
    slot0, eorder0, bc0, K0 = _deal_blocks(dst0, N1, NBLK0 * NCORES)
    slot1, eorder1, bc1, K1 = _deal_blocks(dst1, N2, NBLK1 * NCORES)

    ncT, ncA, ncB = _get_programs(K0, K1)

    # ---- launch T ----
    f0pad = np.zeros((T0_ROWS * NCORES, F_IN), np.float32)
    f0pad[:N0] = feat0
    f1pad = np.zeros((T1_ROWS * NCORES, F_IN), np.float32)
    f1pad[:N1] = feat1
    t_maps = []
    for c in range(NCORES):
        t_maps.append({
            "f0": f0pad[c * T0_ROWS:(c + 1) * T0_ROWS],
            "f1": f1pad[c * T1_ROWS:(c + 1) * T1_ROWS],
            "w0full": W0full, "w0ar": W0ar, "ident": _IDENT,
        })
    resT = _run(ncT, t_maps, trace=_trace)
    fs0 = np.concatenate([r["fs0"] for r in resT.results], axis=0)[:N0]
    er0 = np.concatenate([r["er0"] for r in resT.results], axis=0)[:N1]

    # ---- launch A ----
    stream0, dstr0 = _build_stream(
        fs0[src0], er0[dst0], slot0[dst0], eorder0, bc0, NBLK0 * NCORES, K0)
    a_maps = []
    for c in range(NCORES):
        a_maps.append({
            "stream": stream0[c], "dstr": dstr0[c], "iota": _IOTA,
            "ident": _IDENT, "w1full": W1full,
        })
    resA = _run(ncA, a_maps, trace=_trace)
    h1ext_slots = np.concatenate([r["out"] for r in resA.results], axis=0)
    h1ext = np.zeros((N1, 136), np.float32)
    h1ext[np.arange(N1)] = 0
    # slot0: dst -> global slot; invert
    h1ext[np.arange(N1)] = h1ext_slots[slot0]

    # ---- launch B ----
    er1 = h1ext[map12][:, 132:136]              # [N2, 4]
    stream2, dstr2 = _build_stream(
        h1ext[src1][:, 0:132], er1[dst1], slot1[dst1], eorder1, bc1,
        NBLK1 * NCORES, K1)
    b_maps = []
    for c in range(NCORES):
        b_maps.append({
            "stream": stream2[c], "dstr": dstr2[c], "iota": _IOTA,
        })
    resB = _run(ncB, b_maps, trace=_trace)
    logit_slots = np.concatenate([r["out"] for r in resB.results], axis=0)
    logits = logit_slots[slot1]                 # [N2, 32]

    if _collect_times is not None:
        _collect_times.extend([resT, resA, resB])
    return logits.astype(np.float32)



# revision 2
# speedup vs baseline: 3.5660x; 3.5660x over previous
"""Trainium2 Bass kernel for nn_GATSampling (2-layer bipartite GAT, 8 NeuronCores).

Strategy (SPMD over 8 cores, host re-shards between launches):

  Slot-major edge layout: destination nodes are ranked by degree and dealt
  into blocks of 128 consecutive ranks (so in-block degrees are nearly
  equal); global block gb -> core gb%8, program slot gb//8.  Within a block,
  partition lane p holds the edges of slot p (padded to the block's max
  degree K).  The segment-sum then needs NO one-hot scatter matrix: it is a
  PSUM accumulation of K matmuls against a constant identity stationary.

  Launch T: feature transform with column-stationary fp32 matmuls
            (psum = W0^T @ feat-chunk^T), bf16 outputs for the edge
            streams; el0 via a bf16 matmul against the fs output.
  Host:     gather per-edge rows into per-core streams laid out
            [lane, h, d, k] (k innermost so the DVE runs in 2x mode).
  Launch A: layer-0 edge phase per block: e = el+er (er is per-lane!),
            s = exp(prelu(e)) on Act, a = s/sum_k s (DVE reduce+recip+4x
            tensor_scalar), m = fs*a (DVE 2x), identity-matmul segment
            sum into PSUM, ELU via Act relu/exp decomposition, then
            h1ext = elu @ [W1 | W1@al1m | W1@ar1m] in bf16.
  Launch B: layer-2 edge phase, mean over heads via a strided
            tensor_reduce of the PSUM; outputs logits.

All numeric work except index bookkeeping and the tiny (128x136) weight
products runs on the NeuronCores; streams are bf16, accumulations fp32.
"""
import sys

sys.path.insert(0, "/opt/trn_rl_repo")

import numpy as np
import ml_dtypes

from concourse import bass, mybir, tile, bacc, bass_utils

F32 = mybir.dt.float32
BF16 = mybir.dt.bfloat16
NPBF = ml_dtypes.bfloat16
P = 128
NCORES = 8
NEG_SLOPE = 0.2
H, D = 4, 32
HD = H * D  # 128

# problem sizes (hardcoded per spec)
N0, N1, N2 = 200000, 50000, 12500
E0, E1 = 800000, 200000
F_IN = 128

T0_CHUNKS = 196                 # ceil(N0 / (8*128)) feat0 chunks per core
T0_ROWS = T0_CHUNKS * P         # 25088
T1_CHUNKS = 49                  # feat1 chunks per core
T1_ROWS = T1_CHUNKS * P         # 6272

NBLK0 = 49                      # layer-0 dst blocks per core (49*8*128 >= 50000)
NBLK1 = 13                      # layer-2 dst blocks per core (13*8*128 >= 12500)

EL_PAD = -87.0                  # padding el value: exp(prelu(pad+er)) ~ 1e-8

_cache = {}


# --------------------------------------------------------------------------
# host-side graph partitioning (index bookkeeping only)
# --------------------------------------------------------------------------
def _partition(dst, n_dst, nblk_core):
    """Degree-sorted slot-major partition.

    Returns (order, rank, Khat, start) where rank[node] is the node's
    degree-rank, global block gb = rank // 128 maps to core gb % 8 and
    program block j = gb // 8, lane p = rank % 128.  Khat[j] is the
    (shared across cores) padded edge count per lane for block j, and
    start[j] its chunk offset (C = start[-1]).
    """
    nb = nblk_core * NCORES
    deg = np.bincount(dst, minlength=n_dst)
    order = np.argsort(-deg, kind="stable")
    rank = np.empty(n_dst, np.int64)
    rank[order] = np.arange(n_dst)
    degs = deg[order]
    Khat = np.empty(nblk_core, np.int64)
    for j in range(nblk_core):
        lo = (j * NCORES) * P
        Khat[j] = degs[lo] if lo < n_dst else 1
    Khat = np.maximum(Khat, 1)
    start = np.zeros(nblk_core + 1, np.int64)
    np.cumsum(Khat, out=start[1:])
    return order, rank, Khat, start


def _edge_place(dst, rank, start):
    """Per-edge stream coordinates: (core, chunk, lane)."""
    r = rank[dst]
    eorder = np.argsort(r, kind="stable")
    rs = r[eorder]
    # k = index of edge within its dst's run
    first = np.searchsorted(rs, rs)  # first occurrence index of each value
    k = np.arange(len(rs)) - first
    gb = rs >> 7
    core = gb & (NCORES - 1)
    j = gb >> 3
    lane = rs & (P - 1)
    chunk = start[j] + k
    return eorder, core, chunk, lane


def _build_streams(fs_rows_u16, el_rows, src, eorder, core, chunk, lane,
                   Khat, start, nblk_core):
    """Build per-core sfs (bf16-as-u16) and sel (f32) stream tensors."""
    C = int(start[-1])
    arr_fs = np.zeros((NCORES, C, P, HD), np.uint16)
    arr_el = np.full((NCORES, C, P, H), EL_PAD, np.float32)
    se = src[eorder]
    arr_fs[core, chunk, lane] = fs_rows_u16[se]
    arr_el[core, chunk, lane] = el_rows[se]
    sfs = np.empty((NCORES, P, HD * C), np.uint16)
    sel = np.empty((NCORES, P, H * C), np.float32)
    for j in range(nblk_core):
        s0, K = int(start[j]), int(Khat[j])
        # [k, p, (h d)] -> [p, (h d), k]
        fslab = arr_fs[:, s0:s0 + K].transpose(0, 2, 3, 1)
        sfs[:, :, HD * s0:HD * (s0 + K)] = fslab.reshape(NCORES, P, HD * K)
        eslab = arr_el[:, s0:s0 + K].transpose(0, 2, 3, 1)
        sel[:, :, H * s0:H * (s0 + K)] = eslab.reshape(NCORES, P, H * K)
    return sfs, sel


def _per_slot_table(vals_by_rank, nblk_core, n_dst):
    """vals_by_rank [nslots, 4] -> per-core [P, nblk*4] table."""
    nb = nblk_core * NCORES
    v = np.zeros((nb * P, H), np.float32)
    v[:len(vals_by_rank)] = vals_by_rank
    v = v.reshape(nblk_core, NCORES, P, H)      # [j, core, p, h]
    return np.ascontiguousarray(v.transpose(1, 2, 0, 3)).reshape(
        NCORES, P, nblk_core * H)


# --------------------------------------------------------------------------
# bass programs
# --------------------------------------------------------------------------
def _build_T():
    GC = 4          # chunks per psum batch
    ST = 28         # chunks per dma stage
    nc = bacc.Bacc("TRN2", target_bir_lowering=False, debug=False)
    f0T = nc.dram_tensor("f0T", [P, T0_CHUNKS * P], F32, kind="ExternalInput").ap()
    f1T = nc.dram_tensor("f1T", [P, T1_CHUNKS * P], F32, kind="ExternalInput").ap()
    w0 = nc.dram_tensor("w0", [F_IN, HD], F32, kind="ExternalInput").ap()
    w0ar = nc.dram_tensor("w0ar", [F_IN, H], F32, kind="ExternalInput").ap()
    al0b = nc.dram_tensor("al0b", [HD, H], BF16, kind="ExternalInput").ap()
    fs0bT = nc.dram_tensor("fs0bT", [P, T0_ROWS], BF16, kind="ExternalOutput").ap()
    el0T = nc.dram_tensor("el0T", [H, T0_ROWS], F32, kind="ExternalOutput").ap()
    er0T = nc.dram_tensor("er0T", [H, T1_ROWS], F32, kind="ExternalOutput").ap()

    with tile.TileContext(nc) as tc:
        with (
            tc.tile_pool(name="const", bufs=1) as cpool,
            tc.tile_pool(name="load", bufs=2) as lpool,
            tc.tile_pool(name="fsout", bufs=2) as fpool,
            tc.tile_pool(name="elout", bufs=2) as epool,
            tc.tile_pool(name="erout", bufs=1) as rpool,
            tc.tile_pool(name="ps", bufs=2, space="PSUM") as ppool,
            tc.tile_pool(name="pse", bufs=2, space="PSUM") as ppool2,
        ):
            w0_sb = cpool.tile([F_IN, HD], F32)
            nc.sync.dma_start(w0_sb[:], w0)
            w0ar_sb = cpool.tile([F_IN, H], F32)
            nc.sync.dma_start(w0ar_sb[:], w0ar)
            al0b_sb = cpool.tile([HD, H], BF16)
            nc.sync.dma_start(al0b_sb[:], al0b)

            # ---- feat0 pass: fs0bT + el0T ----
            nstage = T0_CHUNKS // ST  # 7
            for st in range(nstage):
                ld = lpool.tile([P, ST * P], F32, tag="ld")
                nc.sync.dma_start(ld[:], f0T[:, st * ST * P:(st + 1) * ST * P])
                fst = fpool.tile([P, ST * P], BF16, tag="fst")
                elt = epool.tile([H, ST * P], F32, tag="elt")
                for g in range(ST // GC):
                    ps = ppool.tile([P, GC * P], F32, space="PSUM", tag="ps")
                    for c in range(GC):
                        nc.tensor.matmul(
                            ps[:, c * P:(c + 1) * P], lhsT=w0_sb[:],
                            rhs=ld[:, (g * GC + c) * P:(g * GC + c + 1) * P],
                            start=True, stop=True)
                    # fs (bf16) out
                    nc.scalar.copy(fst[:, g * GC * P:(g + 1) * GC * P], ps[:])
                    # el = al0^T @ fs_bf16
                    pse = ppool2.tile([H, GC * P], F32, space="PSUM", tag="pse")
                    nc.tensor.matmul(
                        pse[:], lhsT=al0b_sb[:],
                        rhs=fst[:, g * GC * P:(g + 1) * GC * P],
                        start=True, stop=True)
                    nc.vector.tensor_scalar(
                        out=elt[:, g * GC * P:(g + 1) * GC * P], in0=pse[:],
                        scalar1=1.0, scalar2=None, op0=mybir.AluOpType.mult)
                nc.sync.dma_start(
                    fs0bT[:, st * ST * P:(st + 1) * ST * P], fst[:])
                nc.sync.dma_start(
                    el0T[:, st * ST * P:(st + 1) * ST * P], elt[:])

            # ---- feat1 pass: er0T ----
            ert = rpool.tile([H, T1_ROWS], F32)
            for st in range(2):
                c0 = st * ST
                c1 = min(T1_CHUNKS, (st + 1) * ST)
                ld = lpool.tile([P, ST * P], F32, tag="ld")
                nc.sync.dma_start(ld[:, 0:(c1 - c0) * P],
                                  f1T[:, c0 * P:c1 * P])
                for g in range((c1 - c0 + GC - 1) // GC):
                    g0 = g * GC
                    g1 = min(c1 - c0, g0 + GC)
                    pse = ppool2.tile([H, GC * P], F32, space="PSUM", tag="pse")
                    nc.tensor.matmul(
                        pse[:, 0:(g1 - g0) * P], lhsT=w0ar_sb[:],
                        rhs=ld[:, g0 * P:g1 * P], start=True, stop=True)
                    nc.vector.tensor_scalar(
                        out=ert[:, (c0 + g0) * P:(c0 + g1) * P],
                        in0=pse[:, 0:(g1 - g0) * P],
                        scalar1=1.0, scalar2=None, op0=mybir.AluOpType.mult)
            nc.sync.dma_start(er0T[:], ert[:])

    nc.compile()
    return nc


def _edge_block(nc, Gv, Ev, er_b, wpool, ppool, K):
    """Shared edge-phase core: returns psum_m [P, HD] f32 (unnormalized
    attention-weighted sums; normalization folded into a)."""
    # e = el + er (er is per-lane constant)
    et = wpool.tile([P, H, K], F32, tag="et")
    nc.vector.tensor_tensor(out=et[:], in0=Ev,
                            in1=er_b, op=mybir.AluOpType.add)
    # s = exp(prelu(e))  (padding el=-87 -> s ~ 1e-8)
    lr = wpool.tile([P, H, K], F32, tag="lr")
    nc.scalar.activation(out=lr[:], in_=et[:],
                         func=mybir.ActivationFunctionType.Prelu,
                         alpha=NEG_SLOPE)
    s = wpool.tile([P, H, K], BF16, tag="s")
    nc.scalar.activation(out=s[:], in_=lr[:],
                         func=mybir.ActivationFunctionType.Exp)
    # a = s / sum_k s
    ssum = wpool.tile([P, H], F32, tag="ssum")
    nc.vector.tensor_reduce(out=ssum[:], in_=s[:],
                            axis=mybir.AxisListType.X,
                            op=mybir.AluOpType.add)
    rec = wpool.tile([P, H], F32, tag="rec")
    nc.vector.reciprocal(rec[:], ssum[:])
    a = wpool.tile([P, H, K], BF16, tag="a")
    for h in range(H):
        nc.vector.tensor_scalar(out=a[:, h, :], in0=s[:, h, :],
                                scalar1=rec[:, h:h + 1], scalar2=None,
                                op0=mybir.AluOpType.mult)
    # m = fs * a (in place, 2x mode: k innermost)
    nc.vector.tensor_tensor(out=Gv, in0=Gv,
                            in1=a[:].unsqueeze(2).to_broadcast([P, H, D, K]),
                            op=mybir.AluOpType.mult)
    return rec


def _build_A(Khat):
    nblk = len(Khat)
    start = np.zeros(nblk + 1, np.int64)
    np.cumsum(Khat, out=start[1:])
    C = int(start[-1])
    GRP = 7
    gw = [int(start[min(nblk, (g + 1) * GRP)] - start[g * GRP])
          for g in range((nblk + GRP - 1) // GRP)]
    maxw = max(gw)

    nc = bacc.Bacc("TRN2", target_bir_lowering=False, debug=False)
    sfs = nc.dram_tensor("sfs", [P, HD * C], BF16, kind="ExternalInput").ap()
    sel = nc.dram_tensor("sel", [P, H * C], F32, kind="ExternalInput").ap()
    ers = nc.dram_tensor("ers", [P, nblk * H], F32, kind="ExternalInput").ap()
    identb = nc.dram_tensor("identb", [P, P], BF16, kind="ExternalInput").ap()
    w1b = nc.dram_tensor("w1b", [HD, 136], BF16, kind="ExternalInput").ap()
    obfs = nc.dram_tensor("obfs", [P, nblk * HD], BF16, kind="ExternalOutput").ap()
    obf8 = nc.dram_tensor("obf8", [P, nblk * 8], F32, kind="ExternalOutput").ap()

    with tile.TileContext(nc) as tc:
        with (
            tc.tile_pool(name="const", bufs=1) as cpool,
            tc.tile_pool(name="gload", bufs=2) as gpool,
            tc.tile_pool(name="eload", bufs=2) as epool,
            tc.tile_pool(name="work", bufs=3) as wpool,
            tc.tile_pool(name="stage", bufs=1) as spool,
            tc.tile_pool(name="psm", bufs=2, space="PSUM") as ppool,
            tc.tile_pool(name="pst", bufs=2, space="PSUM") as ppool2,
            tc.tile_pool(name="ps3", bufs=2, space="PSUM") as ppool3,
        ):
            identb_sb = cpool.tile([P, P], BF16)
            nc.sync.dma_start(identb_sb[:], identb)
            w1b_sb = cpool.tile([HD, 136], BF16)
            nc.sync.dma_start(w1b_sb[:], w1b)
            ers_sb = cpool.tile([P, nblk * H], F32)
            nc.sync.dma_start(ers_sb[:], ers)
            ofs_st = spool.tile([P, nblk * HD], BF16)
            of8_st = spool.tile([P, nblk * 8], F32)

            for g in range(len(gw)):
                j0 = g * GRP
                j1 = min(nblk, j0 + GRP)
                s0 = int(start[j0])
                w = gw[g]
                Gg = gpool.tile([P, HD * maxw], BF16, tag="G")
                nc.sync.dma_start(Gg[:, 0:HD * w],
                                  sfs[:, HD * s0:HD * (s0 + w)])
                Eg = epool.tile([P, H * maxw], F32, tag="E")
                nc.sync.dma_start(Eg[:, 0:H * w],
                                  sel[:, H * s0:H * (s0 + w)])
                for j in range(j0, j1):
                    K = int(Khat[j])
                    off = int(start[j]) - s0
                    Gv = Gg[:, HD * off:HD * (off + K)].rearrange(
                        "p (h d k) -> p h d k", h=H, d=D)
                    Ev = Eg[:, H * off:H * (off + K)].rearrange(
                        "p (h k) -> p h k", h=H)
                    er_b = ers_sb[:, j * H:(j + 1) * H].unsqueeze(
                        2).to_broadcast([P, H, K])
                    _edge_block(nc, Gv, Ev, er_b, wpool, ppool, K)
                    psm = ppool.tile([P, HD], F32, space="PSUM", tag="psm")
                    for k in range(K):
                        nc.tensor.matmul(psm[:], lhsT=identb_sb[:],
                                         rhs=Gv[:, :, :, k],
                                         start=(k == 0), stop=(k == K - 1))
                    # elu = relu(x) + exp(-relu(-x)) - 1  (bf16)
                    r1 = wpool.tile([P, HD], BF16, tag="r1")
                    nc.scalar.activation(out=r1[:], in_=psm[:],
                                         func=mybir.ActivationFunctionType.Relu)
                    ng = wpool.tile([P, HD], F32, tag="ng")
                    nc.scalar.activation(out=ng[:], in_=psm[:],
                                         func=mybir.ActivationFunctionType.Relu,
                                         scale=-1.0)
                    ex = wpool.tile([P, HD], BF16, tag="ex")
                    nc.scalar.activation(out=ex[:], in_=ng[:],
                                         func=mybir.ActivationFunctionType.Exp,
                                         scale=-1.0)
                    elu = wpool.tile([P, HD], BF16, tag="elu")
                    nc.vector.tensor_tensor(out=elu[:], in0=r1[:], in1=ex[:],
                                            op=mybir.AluOpType.add)
                    nc.vector.tensor_scalar(out=elu[:], in0=elu[:],
                                            scalar1=1.0, scalar2=None,
                                            op0=mybir.AluOpType.subtract)
                    # h1ext = elu @ [W1 | W1@al1m | W1@ar1m]
                    pst = ppool2.tile([P, P], BF16, space="PSUM", tag="pst")
                    nc.tensor.transpose(out=pst[:], in_=elu[:],
                                        identity=identb_sb[:])
                    eluT = wpool.tile([P, P], BF16, tag="eluT")
                    nc.scalar.copy(eluT[:], pst[:])
                    ps3 = ppool3.tile([P, 136], F32, space="PSUM", tag="ps3")
                    nc.tensor.matmul(ps3[:], lhsT=eluT[:], rhs=w1b_sb[:],
                                     start=True, stop=True)
                    nc.scalar.copy(ofs_st[:, j * HD:(j + 1) * HD],
                                   ps3[:, 0:HD])
                    nc.scalar.copy(of8_st[:, j * 8:(j + 1) * 8],
                                   ps3[:, HD:HD + 8])
            nc.sync.dma_start(obfs, ofs_st[:])
            nc.sync.dma_start(obf8, of8_st[:])

    nc.compile()
    return nc


def _build_B(Khat):
    nblk = len(Khat)
    start = np.zeros(nblk + 1, np.int64)
    np.cumsum(Khat, out=start[1:])
    C = int(start[-1])
    GRP = 5
    gidx = [(0, 5), (5, 9), (9, 13)]
    maxw = max(int(start[b] - start[a]) for a, b in gidx)

    nc = bacc.Bacc("TRN2", target_bir_lowering=False, debug=False)
    sfs = nc.dram_tensor("sfs", [P, HD * C], BF16, kind="ExternalInput").ap()
    sel = nc.dram_tensor("sel", [P, H * C], F32, kind="ExternalInput").ap()
    ers = nc.dram_tensor("ers", [P, nblk * H], F32, kind="ExternalInput").ap()
    identb = nc.dram_tensor("identb", [P, P], BF16, kind="ExternalInput").ap()
    olog = nc.dram_tensor("olog", [P, nblk * D], F32, kind="ExternalOutput").ap()

    with tile.TileContext(nc) as tc:
        with (
            tc.tile_pool(name="const", bufs=1) as cpool,
            tc.tile_pool(name="gload", bufs=2) as gpool,
            tc.tile_pool(name="eload", bufs=2) as epool,
            tc.tile_pool(name="work", bufs=3) as wpool,
            tc.tile_pool(name="stage", bufs=1) as spool,
            tc.tile_pool(name="psm", bufs=2, space="PSUM") as ppool,
        ):
            identb_sb = cpool.tile([P, P], BF16)
            nc.sync.dma_start(identb_sb[:], identb)
            ers_sb = cpool.tile([P, nblk * H], F32)
            nc.sync.dma_start(ers_sb[:], ers)
            olog_st = spool.tile([P, nblk * D], F32)

            for (j0, j1) in gidx:
                s0 = int(start[j0])
                w = int(start[j1]) - s0
                Gg = gpool.tile([P, HD * maxw], BF16, tag="G")
                nc.sync.dma_start(Gg[:, 0:HD * w],
                                  sfs[:, HD * s0:HD * (s0 + w)])
                Eg = epool.tile([P, H * maxw], F32, tag="E")
                nc.sync.dma_start(Eg[:, 0:H * w],
                                  sel[:, H * s0:H * (s0 + w)])
                for j in range(j0, j1):
                    K = int(Khat[j])
                    off = int(start[j]) - s0
                    Gv = Gg[:, HD * off:HD * (off + K)].rearrange(
                        "p (h d k) -> p h d k", h=H, d=D)
                    Ev = Eg[:, H * off:H * (off + K)].rearrange(
                        "p (h k) -> p h k", h=H)
                    er_b = ers_sb[:, j * H:(j + 1) * H].unsqueeze(
                        2).to_broadcast([P, H, K])
                    # normalization includes the head-mean 1/4
                    et = wpool.tile([P, H, K], F32, tag="et")
                    nc.vector.tensor_tensor(out=et[:], in0=Ev, in1=er_b,
                                            op=mybir.AluOpType.add)
                    lr = wpool.tile([P, H, K], F32, tag="lr")
                    nc.scalar.activation(
                        out=lr[:], in_=et[:],
                        func=mybir.ActivationFunctionType.Prelu,
                        alpha=NEG_SLOPE)
                    s = wpool.tile([P, H, K], BF16, tag="s")
                    nc.scalar.activation(
                        out=s[:], in_=lr[:],
                        func=mybir.ActivationFunctionType.Exp)
                    ssum = wpool.tile([P, H], F32, tag="ssum")
                    nc.vector.tensor_reduce(out=ssum[:], in_=s[:],
                                            axis=mybir.AxisListType.X,
                                            op=mybir.AluOpType.add)
                    ssum4 = wpool.tile([P, H], F32, tag="ssum4")
                    nc.vector.tensor_scalar(out=ssum4[:], in0=ssum[:],
                                            scalar1=4.0, scalar2=None,
                                            op0=mybir.AluOpType.mult)
                    rec = wpool.tile([P, H], F32, tag="rec")
                    nc.vector.reciprocal(rec[:], ssum4[:])
                    a = wpool.tile([P, H, K], BF16, tag="a")
                    for h in range(H):
                        nc.vector.tensor_scalar(
                            out=a[:, h, :], in0=s[:, h, :],
                            scalar1=rec[:, h:h + 1], scalar2=None,
                            op0=mybir.AluOpType.mult)
                    nc.vector.tensor_tensor(
                        out=Gv, in0=Gv,
                        in1=a[:].unsqueeze(2).to_broadcast([P, H, D, K]),
                        op=mybir.AluOpType.mult)
                    psm = ppool.tile([P, HD], F32, space="PSUM", tag="psm")
                    for k in range(K):
                        nc.tensor.matmul(psm[:], lhsT=identb_sb[:],
                                         rhs=Gv[:, :, :, k],
                                         start=(k == 0), stop=(k == K - 1))
                    # logits = sum_h psum[:, h, :] (1/4 already in a)
                    nc.vector.tensor_reduce(
                        out=olog_st[:, j * D:(j + 1) * D],
                        in_=psm[:].rearrange("p (h d) -> p d h", h=H),
                        axis=mybir.AxisListType.X,
                        op=mybir.AluOpType.add)
            nc.sync.dma_start(olog, olog_st[:])

    nc.compile()
    return nc


def _get_programs(Khat0, Khat1):
    key = (tuple(Khat0), tuple(Khat1))
    if key not in _cache:
        _cache[key] = (_build_T(), _build_A(Khat0), _build_B(Khat1))
    return _cache[key]


def _run(nc, in_maps, trace=False):
    return bass_utils.run_bass_kernel_spmd(
        nc, in_maps, list(range(NCORES)), trace=trace)


# --------------------------------------------------------------------------
# main entry
# --------------------------------------------------------------------------
def kernel(feat0, feat1, src0, dst0, src1, dst1, map12,
           W0, al0, ar0, W1, al1, ar1, _collect_times=None, _trace=False):
    feat0 = np.asarray(feat0, np.float32)
    feat1 = np.asarray(feat1, np.float32)
    src0 = np.asarray(src0).astype(np.int64)
    dst0 = np.asarray(dst0).astype(np.int64)
    src1 = np.asarray(src1).astype(np.int64)
    dst1 = np.asarray(dst1).astype(np.int64)
    map12 = np.asarray(map12).astype(np.int64)
    W0 = np.asarray(W0, np.float32)
    W1 = np.asarray(W1, np.float32)
    al0 = np.asarray(al0, np.float32); ar0 = np.asarray(ar0, np.float32)
    al1 = np.asarray(al1, np.float32); ar1 = np.asarray(ar1, np.float32)

    # tiny weight products (host)
    al0m = np.zeros((HD, H), np.float32)
    ar0m = np.zeros((HD, H), np.float32)
    al1m = np.zeros((HD, H), np.float32)
    ar1m = np.zeros((HD, H), np.float32)
    for h in range(H):
        al0m[h * D:(h + 1) * D, h] = al0[h]
        ar0m[h * D:(h + 1) * D, h] = ar0[h]
        al1m[h * D:(h + 1) * D, h] = al1[h]
        ar1m[h * D:(h + 1) * D, h] = ar1[h]
    W0ar = (W0 @ ar0m).astype(np.float32)
    W1full_b = np.concatenate(
        [W1, W1 @ al1m, W1 @ ar1m], axis=1).astype(NPBF)
    ident_b = np.eye(P, dtype=NPBF)

    # graph partitioning (host, index-only)
    order0, rank0, Khat0, start0 = _partition(dst0, N1, NBLK0)
    order1, rank1, Khat1, start1 = _partition(dst1, N2, NBLK1)

    ncT, ncA, ncB = _get_programs(Khat0, Khat1)

    # ---- launch T ----
    f0pad = np.zeros((NCORES * T0_ROWS, F_IN), np.float32)
    f0pad[:N0] = feat0
    f1pad = np.zeros((NCORES * T1_ROWS, F_IN), np.float32)
    f1pad[:N1] = feat1
    t_maps = []
    for c in range(NCORES):
        t_maps.append({
            "f0T": np.ascontiguousarray(
                f0pad[c * T0_ROWS:(c + 1) * T0_ROWS].T),
            "f1T": np.ascontiguousarray(
                f1pad[c * T1_ROWS:(c + 1) * T1_ROWS].T),
            "w0": W0, "w0ar": W0ar, "al0b": al0m.astype(NPBF),
        })
    resT = _run(ncT, t_maps, trace=_trace)
    fs0_rows_u16 = np.concatenate(
        [np.asarray(r["fs0bT"]).view(np.uint16).T for r in resT.results])
    el0_rows = np.concatenate([np.asarray(r["el0T"]).T for r in resT.results])
    er0_rows = np.concatenate([np.asarray(r["er0T"]).T for r in resT.results])

    # ---- launch A ----
    eo0, ec0, ech0, el0l = _edge_place(dst0, rank0, start0)
    sfs0, sel0 = _build_streams(fs0_rows_u16, el0_rows, src0,
                                eo0, ec0, ech0, el0l, Khat0, start0, NBLK0)
    er0s = _per_slot_table(er0_rows[order0], NBLK0, N1)
    a_maps = []
    for c in range(NCORES):
        a_maps.append({
            "sfs": sfs0[c].view(NPBF), "sel": sel0[c], "ers": er0s[c],
            "identb": ident_b, "w1b": W1full_b,
        })
    resA = _run(ncA, a_maps, trace=_trace)
    # by-rank tables for layer-1 node features
    nslot0 = NBLK0 * NCORES * P
    fs1_by_rank = np.empty((nslot0, HD), np.uint16)
    f8_by_rank = np.empty((nslot0, 8), np.float32)
    rr = (np.arange(NBLK0)[:, None, None] * NCORES * P
          + np.arange(P)[None, None, :])  # [j, 1, p]
    for c in range(NCORES):
        ranks = (rr + c * P).reshape(-1)  # [(j p)]
        ob = np.asarray(resA.results[c]["obfs"]).view(np.uint16)
        fs1_by_rank[ranks] = ob.reshape(P, NBLK0, HD).transpose(
            1, 0, 2).reshape(-1, HD)
        o8 = np.asarray(resA.results[c]["obf8"])
        f8_by_rank[ranks] = o8.reshape(P, NBLK0, 8).transpose(
            1, 0, 2).reshape(-1, 8)

    # ---- launch B ----
    r0 = rank0  # layer-1 node -> rank
    eo1, ec1, ech1, el1l = _edge_place(dst1, rank1, start1)
    sfs1, sel1 = _build_streams(
        fs1_by_rank, f8_by_rank[:, 0:4], r0[src1],
        eo1, ec1, ech1, el1l, Khat1, start1, NBLK1)
    er1_for_slot = f8_by_rank[r0[map12[order1]]][:, 4:8]  # by layer-2 rank
    er1s = _per_slot_table(er1_for_slot, NBLK1, N2)
    b_maps = []
    for c in range(NCORES):
        b_maps.append({
            "sfs": sfs1[c].view(NPBF), "sel": sel1[c], "ers": er1s[c],
            "identb": ident_b,
        })
    resB = _run(ncB, b_maps, trace=_trace)
    nslot1 = NBLK1 * NCORES * P
    log_by_rank = np.empty((nslot1, D), np.float32)
    rr1 = (np.arange(NBLK1)[:, None, None] * NCORES * P
           + np.arange(P)[None, None, :])
    for c in range(NCORES):
        ranks = (rr1 + c * P).reshape(-1)
        ol = np.asarray(resB.results[c]["olog"])
        log_by_rank[ranks] = ol.reshape(P, NBLK1, D).transpose(
            1, 0, 2).reshape(-1, D)
    logits = log_by_rank[rank1[np.arange(N2)]]

    if _collect_times is not None:
        _collect_times.extend([resT, resA, resB])
    return logits.astype(np.float32)


# revision 4
# speedup vs baseline: 4.7642x; 1.3360x over previous
"""Trainium2 Bass kernel for nn_GATSampling (2-layer bipartite GAT, 8 NeuronCores).

Strategy (SPMD over 8 cores, host re-shards between launches):

  Slot-major edge layout: destination nodes are ranked by degree and dealt
  into blocks of 128 consecutive ranks (so in-block degrees are nearly
  equal); global block gb -> core gb%8, program slot gb//8.  Within a block,
  partition lane p holds the edges of slot p (padded to the block's max
  degree K).  The segment-sum then needs NO one-hot scatter matrix: it is a
  PSUM accumulation of matmuls against a constant identity stationary.
  Four k-chunks go into one 512-col matmul (keeps the moving operand's
  last dim packed); a single DVE tensor_reduce folds the quad lanes.

  Launch T: feature transform with column-stationary float32r matmuls
            (psum = W^T @ chunk^T, 512 cols per matmul), bf16 fs/el f32
            outputs for the edge streams.
  Host:     gather per-edge rows into per-core streams laid out
            [lane, h, d, k] (k innermost so the DVE multiply runs in 2x).
  Launch A: layer-0 edge phase per block: e = el+er (er is per-lane),
            s = exp(prelu(e)) on Act, a = s * (1/sum_k s) broadcast,
            m = fs*a (DVE 2x), identity-matmul segment sum, ELU via Act
            relu/exp decomposition (-1 folded into the transpose copy's
            bias), h1ext = elu @ [W1 | W1@al1m | W1@ar1m] in bf16.
  Launch B: layer-2 edge phase; quad-fold and mean over heads fused into
            one strided tensor_reduce; outputs logits.

All numeric work except index bookkeeping and the tiny (128x136) weight
products runs on the NeuronCores; streams are bf16, accumulations fp32.
"""
import sys

sys.path.insert(0, "/opt/trn_rl_repo")

import numpy as np
import ml_dtypes

from concourse import bass, mybir, tile, bacc, bass_utils

F32 = mybir.dt.float32
F32R = mybir.dt.float32r
BF16 = mybir.dt.bfloat16
NPBF = ml_dtypes.bfloat16
P = 128
NCORES = 8
NEG_SLOPE = 0.2
H, D = 4, 32
HD = H * D  # 128
PARW = 4    # k-chunks folded into one 512-col matmul

# problem sizes (hardcoded per spec)
N0, N1, N2 = 200000, 50000, 12500
E0, E1 = 800000, 200000
F_IN = 128

T0_CHUNKS = 196                 # ceil(N0 / (8*128)) feat0 chunks per core
T0_ROWS = T0_CHUNKS * P         # 25088
T1_CHUNKS = 49                  # feat1 chunks per core
T1_ROWS = T1_CHUNKS * P         # 6272

NBLK0 = 49                      # layer-0 dst blocks per core
NBLK1 = 13                      # layer-2 dst blocks per core

EL_PAD = -87.0                  # padding el: exp(prelu(pad+er)) ~ 1e-8

_cache = {}


# --------------------------------------------------------------------------
# host-side graph partitioning (index bookkeeping only)
# --------------------------------------------------------------------------
def _partition(dst, n_dst, nblk_core):
    nb = nblk_core * NCORES
    deg = np.bincount(dst, minlength=n_dst)
    order = np.argsort(-deg, kind="stable")
    rank = np.empty(n_dst, np.int64)
    rank[order] = np.arange(n_dst)
    degs = deg[order]
    Khat = np.empty(nblk_core, np.int64)
    for j in range(nblk_core):
        lo = (j * NCORES) * P
        Khat[j] = degs[lo] if lo < n_dst else 1
    Khat = np.maximum(Khat, PARW)
    start = np.zeros(nblk_core + 1, np.int64)
    np.cumsum(Khat, out=start[1:])
    return order, rank, Khat, start


def _edge_place(dst, rank, start):
    """Per-edge stream coordinates: (core, chunk, lane)."""
    r = rank[dst]
    eorder = np.argsort(r, kind="stable")
    rs = r[eorder]
    first = np.searchsorted(rs, rs)
    k = np.arange(len(rs)) - first
    gb = rs >> 7
    core = gb & (NCORES - 1)
    j = gb >> 3
    lane = rs & (P - 1)
    chunk = start[j] + k
    return eorder, core, chunk, lane


def _build_streams(fs_rows_u16, el_rows, src, eorder, core, chunk, lane,
                   Khat, start, nblk_core):
    C = int(start[-1])
    arr_fs = np.zeros((NCORES, C, P, HD), np.uint16)
    arr_el = np.full((NCORES, C, P, H), EL_PAD, np.float32)
    se = src[eorder]
    arr_fs[core, chunk, lane] = fs_rows_u16[se]
    arr_el[core, chunk, lane] = el_rows[se]
    sfs = np.empty((NCORES, P, HD * C), np.uint16)
    sel = np.empty((NCORES, P, H * C), np.float32)
    for j in range(nblk_core):
        s0, K = int(start[j]), int(Khat[j])
        fslab = arr_fs[:, s0:s0 + K].transpose(0, 2, 3, 1)
        sfs[:, :, HD * s0:HD * (s0 + K)] = fslab.reshape(NCORES, P, HD * K)
        eslab = arr_el[:, s0:s0 + K].transpose(0, 2, 3, 1)
        sel[:, :, H * s0:H * (s0 + K)] = eslab.reshape(NCORES, P, H * K)
    return sfs, sel


def _per_slot_table(vals_by_rank, nblk_core):
    nb = nblk_core * NCORES
    v = np.zeros((nb * P, H), np.float32)
    v[:len(vals_by_rank)] = vals_by_rank
    v = v.reshape(nblk_core, NCORES, P, H)
    return np.ascontiguousarray(v.transpose(1, 2, 0, 3)).reshape(
        NCORES, P, nblk_core * H)


# --------------------------------------------------------------------------
# bass programs
# --------------------------------------------------------------------------
def _build_T():
    GC = 4          # chunks per matmul batch (512 cols)
    ST = 28         # chunks per dma stage
    nc = bacc.Bacc("TRN2", target_bir_lowering=False, debug=False)
    f0T = nc.dram_tensor("f0T", [P, T0_CHUNKS * P], F32R, kind="ExternalInput").ap()
    f1T = nc.dram_tensor("f1T", [P, T1_CHUNKS * P], F32R, kind="ExternalInput").ap()
    w0 = nc.dram_tensor("w0", [F_IN, HD], F32R, kind="ExternalInput").ap()
    w0al = nc.dram_tensor("w0al", [F_IN, H], F32R, kind="ExternalInput").ap()
    w0ar = nc.dram_tensor("w0ar", [F_IN, H], F32R, kind="ExternalInput").ap()
    fs0bT = nc.dram_tensor("fs0bT", [P, T0_ROWS], BF16, kind="ExternalOutput").ap()
    el0T = nc.dram_tensor("el0T", [H, T0_ROWS], F32, kind="ExternalOutput").ap()
    er0T = nc.dram_tensor("er0T", [H, T1_ROWS], F32, kind="ExternalOutput").ap()

    with tile.TileContext(nc) as tc:
        with (
            tc.tile_pool(name="const", bufs=1) as cpool,
            tc.tile_pool(name="load", bufs=2) as lpool,
            tc.tile_pool(name="fsout", bufs=2) as fpool,
            tc.tile_pool(name="elout", bufs=2) as epool,
            tc.tile_pool(name="erout", bufs=1) as rpool,
            tc.tile_pool(name="ps", bufs=2, space="PSUM") as ppool,
            tc.tile_pool(name="pse", bufs=2, space="PSUM") as ppool2,
        ):
            w0_sb = cpool.tile([F_IN, HD], F32R)
            nc.sync.dma_start(w0_sb[:], w0)
            w0al_sb = cpool.tile([F_IN, H], F32R)
            nc.sync.dma_start(w0al_sb[:], w0al)
            w0ar_sb = cpool.tile([F_IN, H], F32R)
            nc.sync.dma_start(w0ar_sb[:], w0ar)

            # ---- feat0 pass: fs0bT (bf16) + el0T (f32) ----
            for st in range(T0_CHUNKS // ST):
                ld = lpool.tile([P, ST * P], F32R, tag="ld")
                nc.sync.dma_start(ld[:], f0T[:, st * ST * P:(st + 1) * ST * P])
                fst = fpool.tile([P, ST * P], BF16, tag="fst")
                elt = epool.tile([H, ST * P], F32, tag="elt")
                for g in range(ST // GC):
                    sl = slice(g * GC * P, (g + 1) * GC * P)
                    ps = ppool.tile([P, GC * P], F32, space="PSUM", tag="ps")
                    nc.tensor.matmul(ps[:], lhsT=w0_sb[:], rhs=ld[:, sl],
                                     start=True, stop=True)
                    nc.scalar.copy(fst[:, sl], ps[:])
                    pse = ppool2.tile([H, GC * P], F32, space="PSUM", tag="pse")
                    nc.tensor.matmul(pse[:], lhsT=w0al_sb[:], rhs=ld[:, sl],
                                     start=True, stop=True)
                    nc.vector.tensor_scalar(
                        out=elt[:, sl], in0=pse[:], scalar1=1.0,
                        scalar2=None, op0=mybir.AluOpType.mult)
                nc.sync.dma_start(
                    fs0bT[:, st * ST * P:(st + 1) * ST * P], fst[:])
                nc.sync.dma_start(
                    el0T[:, st * ST * P:(st + 1) * ST * P], elt[:])

            # ---- feat1 pass: er0T (f32) ----
            ert = rpool.tile([H, T1_ROWS], F32)
            for st in range(2):
                c0 = st * ST
                c1 = min(T1_CHUNKS, (st + 1) * ST)
                ld = lpool.tile([P, ST * P], F32R, tag="ld")
                nc.sync.dma_start(ld[:, 0:(c1 - c0) * P],
                                  f1T[:, c0 * P:c1 * P])
                for g in range((c1 - c0 + GC - 1) // GC):
                    g0 = g * GC
                    g1 = min(c1 - c0, g0 + GC)
                    pse = ppool2.tile([H, GC * P], F32, space="PSUM", tag="pse")
                    nc.tensor.matmul(
                        pse[:, 0:(g1 - g0) * P], lhsT=w0ar_sb[:],
                        rhs=ld[:, g0 * P:g1 * P], start=True, stop=True)
                    nc.vector.tensor_scalar(
                        out=ert[:, (c0 + g0) * P:(c0 + g1) * P],
                        in0=pse[:, 0:(g1 - g0) * P],
                        scalar1=1.0, scalar2=None, op0=mybir.AluOpType.mult)
            nc.sync.dma_start(er0T, ert[:])

    nc.compile()
    return nc


def _attention(nc, Gv, Ev, er_b, wpool, K, recip_scale):
    """e=el+er, s=exp(prelu(e)), a=s/sum, m=fs*a in place. Returns None."""
    et = wpool.tile([P, H, K], F32, tag="et")
    nc.vector.tensor_tensor(out=et[:], in0=Ev, in1=er_b,
                            op=mybir.AluOpType.add)
    lr = wpool.tile([P, H, K], F32, tag="lr")
    nc.scalar.activation(out=lr[:], in_=et[:],
                         func=mybir.ActivationFunctionType.Prelu,
                         alpha=NEG_SLOPE)
    s = wpool.tile([P, H, K], BF16, tag="s")
    nc.scalar.activation(out=s[:], in_=lr[:],
                         func=mybir.ActivationFunctionType.Exp)
    ssum = wpool.tile([P, H], F32, tag="ssum")
    nc.vector.tensor_reduce(out=ssum[:], in_=s[:],
                            axis=mybir.AxisListType.X,
                            op=mybir.AluOpType.add)
    if recip_scale != 1.0:
        nc.vector.tensor_scalar(out=ssum[:], in0=ssum[:],
                                scalar1=recip_scale, scalar2=None,
                                op0=mybir.AluOpType.mult)
    rec = wpool.tile([P, H], F32, tag="rec")
    nc.vector.reciprocal(rec[:], ssum[:])
    a = wpool.tile([P, H, K], BF16, tag="a")
    nc.vector.tensor_tensor(
        out=a[:], in0=s[:],
        in1=rec[:].unsqueeze(2).to_broadcast([P, H, K]),
        op=mybir.AluOpType.mult)
    nc.vector.tensor_tensor(
        out=Gv, in0=Gv,
        in1=a[:].unsqueeze(2).to_broadcast([P, H, D, K]),
        op=mybir.AluOpType.mult)


def _seg_matmuls(nc, psq, Gv, identb_sb, K):
    """Identity segment-sum into quad psum [P, HD, PARW]."""
    nq = K // PARW
    rem = K - nq * PARW
    Gq = Gv[:, :, :, 0:nq * PARW].rearrange(
        "p h d (kk par) -> p h d kk par", par=PARW)
    for kk in range(nq):
        nc.tensor.matmul(psq[:], lhsT=identb_sb[:],
                         rhs=Gq[:, :, :, kk, :],
                         start=(kk == 0), stop=(kk == nq - 1 and rem == 0))
    for i in range(rem):
        nc.tensor.matmul(psq[:, :, i], lhsT=identb_sb[:],
                         rhs=Gv[:, :, :, nq * PARW + i],
                         start=False, stop=(i == rem - 1))


def _build_A(Khat):
    nblk = len(Khat)
    start = np.zeros(nblk + 1, np.int64)
    np.cumsum(Khat, out=start[1:])
    C = int(start[-1])
    GRP = 7
    gw = [int(start[min(nblk, (g + 1) * GRP)] - start[g * GRP])
          for g in range((nblk + GRP - 1) // GRP)]
    maxw = max(gw)

    nc = bacc.Bacc("TRN2", target_bir_lowering=False, debug=False)
    sfs = nc.dram_tensor("sfs", [P, HD * C], BF16, kind="ExternalInput").ap()
    sel = nc.dram_tensor("sel", [P, H * C], F32, kind="ExternalInput").ap()
    ers = nc.dram_tensor("ers", [P, nblk * H], F32, kind="ExternalInput").ap()
    identb = nc.dram_tensor("identb", [P, P], BF16, kind="ExternalInput").ap()
    w1b = nc.dram_tensor("w1b", [HD, 136], BF16, kind="ExternalInput").ap()
    obfs = nc.dram_tensor("obfs", [P, nblk * HD], BF16, kind="ExternalOutput").ap()
    obf8 = nc.dram_tensor("obf8", [P, nblk * 8], F32, kind="ExternalOutput").ap()

    with tile.TileContext(nc) as tc:
        with (
            tc.tile_pool(name="const", bufs=1) as cpool,
            tc.tile_pool(name="gload", bufs=2) as gpool,
            tc.tile_pool(name="eload", bufs=2) as epool,
            tc.tile_pool(name="work", bufs=3) as wpool,
            tc.tile_pool(name="stage", bufs=1) as spool,
            tc.tile_pool(name="psm", bufs=2, space="PSUM") as ppool,
            tc.tile_pool(name="pst", bufs=2, space="PSUM") as ppool2,
            tc.tile_pool(name="ps3", bufs=2, space="PSUM") as ppool3,
        ):
            identb_sb = cpool.tile([P, P], BF16)
            nc.sync.dma_start(identb_sb[:], identb)
            w1b_sb = cpool.tile([HD, 136], BF16)
            nc.sync.dma_start(w1b_sb[:], w1b)
            ers_sb = cpool.tile([P, nblk * H], F32)
            nc.sync.dma_start(ers_sb[:], ers)
            ofs_st = spool.tile([P, nblk * HD], BF16)
            of8_st = spool.tile([P, nblk * 8], F32)

            for g in range(len(gw)):
                j0 = g * GRP
                j1 = min(nblk, j0 + GRP)
                s0 = int(start[j0])
                w = gw[g]
                Gg = gpool.tile([P, HD * maxw], BF16, tag="G")
                nc.sync.dma_start(Gg[:, 0:HD * w],
                                  sfs[:, HD * s0:HD * (s0 + w)])
                Eg = epool.tile([P, H * maxw], F32, tag="E")
                nc.sync.dma_start(Eg[:, 0:H * w],
                                  sel[:, H * s0:H * (s0 + w)])
                for j in range(j0, j1):
                    K = int(Khat[j])
                    off = int(start[j]) - s0
                    Gv = Gg[:, HD * off:HD * (off + K)].rearrange(
                        "p (h d k) -> p h d k", h=H, d=D)
                    Ev = Eg[:, H * off:H * (off + K)].rearrange(
                        "p (h k) -> p h k", h=H)
                    er_b = ers_sb[:, j * H:(j + 1) * H].unsqueeze(
                        2).to_broadcast([P, H, K])
                    _attention(nc, Gv, Ev, er_b, wpool, K, 1.0)
                    psq = ppool.tile([P, HD, PARW], F32, space="PSUM",
                                     tag="psq")
                    _seg_matmuls(nc, psq, Gv, identb_sb, K)
                    y = wpool.tile([P, HD], F32, tag="y")
                    nc.vector.tensor_reduce(out=y[:], in_=psq[:],
                                            axis=mybir.AxisListType.X,
                                            op=mybir.AluOpType.add)
                    # elu+1 = relu(y) + exp(-relu(-y)); -1 folded into copy
                    r1 = wpool.tile([P, HD], BF16, tag="r1")
                    nc.scalar.activation(out=r1[:], in_=y[:],
                                         func=mybir.ActivationFunctionType.Relu)
                    ng = wpool.tile([P, HD], F32, tag="ng")
                    nc.scalar.activation(out=ng[:], in_=y[:],
                                         func=mybir.ActivationFunctionType.Relu,
                                         scale=-1.0)
                    ex = wpool.tile([P, HD], BF16, tag="ex")
                    nc.scalar.activation(out=ex[:], in_=ng[:],
                                         func=mybir.ActivationFunctionType.Exp,
                                         scale=-1.0)
                    elu1 = wpool.tile([P, HD], BF16, tag="elu1")
                    nc.vector.tensor_tensor(out=elu1[:], in0=r1[:], in1=ex[:],
                                            op=mybir.AluOpType.add)
                    pst = ppool2.tile([P, P], BF16, space="PSUM", tag="pst")
                    nc.tensor.transpose(out=pst[:], in_=elu1[:],
                                        identity=identb_sb[:])
                    eluT = wpool.tile([P, P], BF16, tag="eluT")
                    nc.scalar.activation(out=eluT[:], in_=pst[:],
                                         func=mybir.ActivationFunctionType.Copy,
                                         bias=-1.0)
                    ps3 = ppool3.tile([P, 136], F32, space="PSUM", tag="ps3")
                    nc.tensor.matmul(ps3[:], lhsT=eluT[:], rhs=w1b_sb[:],
                                     start=True, stop=True)
                    nc.scalar.copy(ofs_st[:, j * HD:(j + 1) * HD],
                                   ps3[:, 0:HD])
                    nc.scalar.copy(of8_st[:, j * 8:(j + 1) * 8],
                                   ps3[:, HD:HD + 8])
            nc.sync.dma_start(obfs, ofs_st[:])
            nc.sync.dma_start(obf8, of8_st[:])

    nc.compile()
    return nc


def _build_B(Khat):
    nblk = len(Khat)
    start = np.zeros(nblk + 1, np.int64)
    np.cumsum(Khat, out=start[1:])
    C = int(start[-1])
    gidx = [(0, 5), (5, 9), (9, 13)]
    maxw = max(int(start[b] - start[a]) for a, b in gidx)

    nc = bacc.Bacc("TRN2", target_bir_lowering=False, debug=False)
    sfs = nc.dram_tensor("sfs", [P, HD * C], BF16, kind="ExternalInput").ap()
    sel = nc.dram_tensor("sel", [P, H * C], F32, kind="ExternalInput").ap()
    ers = nc.dram_tensor("ers", [P, nblk * H], F32, kind="ExternalInput").ap()
    identb = nc.dram_tensor("identb", [P, P], BF16, kind="ExternalInput").ap()
    olog = nc.dram_tensor("olog", [P, nblk * D], F32, kind="ExternalOutput").ap()

    with tile.TileContext(nc) as tc:
        with (
            tc.tile_pool(name="const", bufs=1) as cpool,
            tc.tile_pool(name="gload", bufs=2) as gpool,
            tc.tile_pool(name="eload", bufs=2) as epool,
            tc.tile_pool(name="work", bufs=3) as wpool,
            tc.tile_pool(name="stage", bufs=1) as spool,
            tc.tile_pool(name="psm", bufs=2, space="PSUM") as ppool,
        ):
            identb_sb = cpool.tile([P, P], BF16)
            nc.sync.dma_start(identb_sb[:], identb)
            ers_sb = cpool.tile([P, nblk * H], F32)
            nc.sync.dma_start(ers_sb[:], ers)
            olog_st = spool.tile([P, nblk * D], F32)

            for (j0, j1) in gidx:
                s0 = int(start[j0])
                w = int(start[j1]) - s0
                Gg = gpool.tile([P, HD * maxw], BF16, tag="G")
                nc.sync.dma_start(Gg[:, 0:HD * w],
                                  sfs[:, HD * s0:HD * (s0 + w)])
                Eg = epool.tile([P, H * maxw], F32, tag="E")
                nc.sync.dma_start(Eg[:, 0:H * w],
                                  sel[:, H * s0:H * (s0 + w)])
                for j in range(j0, j1):
                    K = int(Khat[j])
                    off = int(start[j]) - s0
                    Gv = Gg[:, HD * off:HD * (off + K)].rearrange(
                        "p (h d k) -> p h d k", h=H, d=D)
                    Ev = Eg[:, H * off:H * (off + K)].rearrange(
                        "p (h k) -> p h k", h=H)
                    er_b = ers_sb[:, j * H:(j + 1) * H].unsqueeze(
                        2).to_broadcast([P, H, K])
                    # 1/4 head-mean folded into the reciprocal scale
                    _attention(nc, Gv, Ev, er_b, wpool, K, 4.0)
                    psq = ppool.tile([P, HD, PARW], F32, space="PSUM",
                                     tag="psq")
                    _seg_matmuls(nc, psq, Gv, identb_sb, K)
                    # logits = sum over (h, par) of quad psum
                    nc.vector.tensor_reduce(
                        out=olog_st[:, j * D:(j + 1) * D],
                        in_=psq[:].rearrange("p (h d) par -> p d h par", h=H),
                        axis=mybir.AxisListType.XY,
                        op=mybir.AluOpType.add)
            nc.sync.dma_start(olog, olog_st[:])

    nc.compile()
    return nc


def _get_programs(Khat0, Khat1):
    key = (tuple(Khat0), tuple(Khat1))
    if key not in _cache:
        _cache[key] = (_build_T(), _build_A(Khat0), _build_B(Khat1))
    return _cache[key]


def _run(nc, in_maps, trace=False):
    return bass_utils.run_bass_kernel_spmd(
        nc, in_maps, list(range(NCORES)), trace=trace)


# --------------------------------------------------------------------------
# main entry
# --------------------------------------------------------------------------
def kernel(feat0, feat1, src0, dst0, src1, dst1, map12,
           W0, al0, ar0, W1, al1, ar1, _collect_times=None, _trace=False):
    feat0 = np.asarray(feat0, np.float32)
    feat1 = np.asarray(feat1, np.float32)
    src0 = np.asarray(src0).astype(np.int64)
    dst0 = np.asarray(dst0).astype(np.int64)
    src1 = np.asarray(src1).astype(np.int64)
    dst1 = np.asarray(dst1).astype(np.int64)
    map12 = np.asarray(map12).astype(np.int64)
    W0 = np.asarray(W0, np.float32)
    W1 = np.asarray(W1, np.float32)
    al0 = np.asarray(al0, np.float32); ar0 = np.asarray(ar0, np.float32)
    al1 = np.asarray(al1, np.float32); ar1 = np.asarray(ar1, np.float32)

    al0m = np.zeros((HD, H), np.float32)
    ar0m = np.zeros((HD, H), np.float32)
    al1m = np.zeros((HD, H), np.float32)
    ar1m = np.zeros((HD, H), np.float32)
    for h in range(H):
        al0m[h * D:(h + 1) * D, h] = al0[h]
        ar0m[h * D:(h + 1) * D, h] = ar0[h]
        al1m[h * D:(h + 1) * D, h] = al1[h]
        ar1m[h * D:(h + 1) * D, h] = ar1[h]
    W0al = (W0 @ al0m).astype(np.float32)
    W0ar = (W0 @ ar0m).astype(np.float32)
    W1full_b = np.concatenate(
        [W1, W1 @ al1m, W1 @ ar1m], axis=1).astype(NPBF)
    ident_b = np.eye(P, dtype=NPBF)

    order0, rank0, Khat0, start0 = _partition(dst0, N1, NBLK0)
    order1, rank1, Khat1, start1 = _partition(dst1, N2, NBLK1)

    ncT, ncA, ncB = _get_programs(Khat0, Khat1)

    # ---- launch T ----
    f0pad = np.zeros((NCORES * T0_ROWS, F_IN), np.float32)
    f0pad[:N0] = feat0
    f1pad = np.zeros((NCORES * T1_ROWS, F_IN), np.float32)
    f1pad[:N1] = feat1
    t_maps = []
    for c in range(NCORES):
        t_maps.append({
            "f0T": np.ascontiguousarray(
                f0pad[c * T0_ROWS:(c + 1) * T0_ROWS].T),
            "f1T": np.ascontiguousarray(
                f1pad[c * T1_ROWS:(c + 1) * T1_ROWS].T),
            "w0": W0, "w0al": W0al, "w0ar": W0ar,
        })
    resT = _run(ncT, t_maps, trace=_trace)
    fs0_rows_u16 = np.concatenate(
        [np.asarray(r["fs0bT"]).view(np.uint16).T for r in resT.results])
    el0_rows = np.concatenate([np.asarray(r["el0T"]).T for r in resT.results])
    er0_rows = np.concatenate([np.asarray(r["er0T"]).T for r in resT.results])

    # ---- launch A ----
    eo0, ec0, ech0, el0l = _edge_place(dst0, rank0, start0)
    sfs0, sel0 = _build_streams(fs0_rows_u16, el0_rows, src0,
                                eo0, ec0, ech0, el0l, Khat0, start0, NBLK0)
    er0s = _per_slot_table(er0_rows[order0], NBLK0)
    a_maps = []
    for c in range(NCORES):
        a_maps.append({
            "sfs": sfs0[c].view(NPBF), "sel": sel0[c], "ers": er0s[c],
            "identb": ident_b, "w1b": W1full_b,
        })
    resA = _run(ncA, a_maps, trace=_trace)
    nslot0 = NBLK0 * NCORES * P
    fs1_by_rank = np.empty((nslot0, HD), np.uint16)
    f8_by_rank = np.empty((nslot0, 8), np.float32)
    rr = (np.arange(NBLK0)[:, None, None] * NCORES * P
          + np.arange(P)[None, None, :])
    for c in range(NCORES):
        ranks = (rr + c * P).reshape(-1)
        ob = np.asarray(resA.results[c]["obfs"]).view(np.uint16)
        fs1_by_rank[ranks] = ob.reshape(P, NBLK0, HD).transpose(
            1, 0, 2).reshape(-1, HD)
        o8 = np.asarray(resA.results[c]["obf8"])
        f8_by_rank[ranks] = o8.reshape(P, NBLK0, 8).transpose(
            1, 0, 2).reshape(-1, 8)

    # ---- launch B ----
    eo1, ec1, ech1, el1l = _edge_place(dst1, rank1, start1)
    sfs1, sel1 = _build_streams(
        fs1_by_rank, f8_by_rank[:, 0:4], rank0[src1],
        eo1, ec1, ech1, el1l, Khat1, start1, NBLK1)
    er1_for_slot = f8_by_rank[rank0[map12[order1]]][:, 4:8]
    er1s = _per_slot_table(er1_for_slot, NBLK1)
    b_maps = []
    for c in range(NCORES):
        b_maps.append({
            "sfs": sfs1[c].view(NPBF), "sel": sel1[c], "ers": er1s[c],
            "identb": ident_b,
        })
    resB = _run(ncB, b_maps, trace=_trace)
    nslot1 = NBLK1 * NCORES * P
    log_by_rank = np.empty((nslot1, D), np.float32)
    rr1 = (np.arange(NBLK1)[:, None, None] * NCORES * P
           + np.arange(P)[None, None, :])
    for c in range(NCORES):
        ranks = (rr1 + c * P).reshape(-1)
        ol = np.asarray(resB.results[c]["olog"])
        log_by_rank[ranks] = ol.reshape(P, NBLK1, D).transpose(
            1, 0, 2).reshape(-1, D)
    logits = log_by_rank[rank1[np.arange(N2)]]

    if _collect_times is not None:
        _collect_times.extend([resT, resA, resB])
    return logits.astype(np.float32)
